# revision 1
# baseline (speedup 1.0000x reference)
"""Trainium2 Bass kernel for nn_CorrectSplineLinear (embedding_lookup regime).

Math: reference computes
    W[o,t,f] = sum_c interp[o,t,c] * E[c,f]        (interp = piecewise-linear in t)
    out[o,b,t] = sum_f x[b,f] * W[o,t,f]
which collapses algebraically to
    y[c,b]    = sum_f E[c,f] * x[b,f]              ([128,128] matmul)
    Z[o,s,b]  = sum_c cv[o,s,c] * y[c,b]           ([128,128] matmul per core)
    out[o,b,t]= Z[o,j(t),b] + tl(t)*(Z[o,j(t)+1,b] - Z[o,j(t),b])
so no [O,I,I] weight is ever materialized.  The kernel is memory-bound on
writing the [256,128,512] fp32 output (8 MiB per core across 8 cores).

Sharding: out_features O=256 split across 8 cores (32 rows each); x and E
replicated; each core gets its control_values slice pre-transposed.
"""

import sys
from contextlib import ExitStack

import numpy as np

try:
    import concourse.bass as bass
except ImportError:  # fresh grading dir: concourse lives in the repo checkout
    sys.path.insert(0, "/opt/trn_rl_repo")
    import concourse.bass as bass

import concourse.bacc as bacc
import concourse.mybir as mybir
import concourse.tile as tile
from concourse.bass_utils import run_bass_kernel_spmd

N_CORES = 8
O, I, K, C, B = 256, 512, 3, 128, 128
OL = O // N_CORES  # 32 output rows per core
NS = K + 1  # 4 control values per output row
F32 = mybir.dt.float32

# ---- spline geometry (input-independent, mirrors reference arithmetic) ----
_t = np.linspace(0.0, 1.0, I).astype(np.float32)
_ts = (_t * np.float32(K)).astype(np.float32)
_j = np.clip(np.floor(_ts), 0.0, float(K - 1)).astype(np.int32)
_TL = (_ts - _j.astype(np.float32)).astype(np.float32)  # [I] local coord in segment
_b0 = int(np.searchsorted(_j, 1))  # first t index in segment 1
_b1 = int(np.searchsorted(_j, 2))  # first t index in segment 2
# Even-length op spans for the DVE 2x fp32 perf mode.  Padded cells are
# overwritten by the middle-segment op, which is issued last on the same
# engine, so the final contents are exact.
_b0e = _b0 + (_b0 & 1)  # segment-0 op covers [0, _b0e)
_s2e = _b1 - (_b1 & 1)  # segment-2 op covers [_s2e, I)
_SPANS = [(0, 0, _b0e), (2, _s2e, I), (1, _b0, _b1)]  # (segment j, t0, t1)

# ---- packed-input column layout ([128, _TOT] fp32, one DMA per core) ----
_XT0 = 0  # x^T as 4 chunks [128f, 128b]
_ET0 = _XT0 + 4 * B  # E^T as 4 chunks [128f, 128c]
_CV0 = _ET0 + 4 * C  # cv slab transposed: [c, o*4+s]
_TL0 = _CV0 + OL * NS  # tl broadcast to 128 partitions
_TOT = _TL0 + I

GROUP = 4  # output rows per store DMA (4*128*512*4B = 1 MiB)
NGRP = OL // GROUP

_cache: dict = {}


def _build_nc():
    nc = bacc.Bacc("TRN2", target_bir_lowering=False, debug=False, num_devices=N_CORES)
    pk_d = nc.dram_tensor("pk", [128, _TOT], F32, kind="ExternalInput")
    out_d = nc.dram_tensor("out", [OL, B, I], F32, kind="ExternalOutput")

    with tile.TileContext(nc) as tc, ExitStack() as ctx:
        constp = ctx.enter_context(tc.tile_pool(name="const", bufs=1))
        psump = ctx.enter_context(
            tc.tile_pool(name="psum", bufs=1, space=bass.MemorySpace.PSUM)
        )
        outp = ctx.enter_context(tc.tile_pool(name="outs", bufs=1))

        pk = constp.tile([128, _TOT], F32)
        nc.sync.dma_start(pk[:], pk_d[:])

        # y[c,b] = sum_f E[c,f] x[b,f]: accumulate over 4 chunks of f.
        y_ps = psump.tile([128, B], F32)
        for k in range(4):
            nc.tensor.matmul(
                y_ps[:],
                pk[:, _ET0 + k * C : _ET0 + (k + 1) * C],  # lhsT [f_chunk, c]
                pk[:, _XT0 + k * B : _XT0 + (k + 1) * B],  # rhs  [f_chunk, b]
                start=(k == 0),
                stop=(k == 3),
            )
        y_sb = constp.tile([128, B], F32)
        nc.vector.tensor_copy(y_sb[:], y_ps[:])

        # ZT[b, o*4+s] = sum_c y[c,b] cvT[c, o*4+s]
        zt_ps = psump.tile([128, OL * NS], F32)
        nc.tensor.matmul(
            zt_ps[:], y_sb[:], pk[:, _CV0 : _CV0 + OL * NS], start=True, stop=True
        )
        zt = constp.tile([128, OL * NS], F32)
        nc.vector.tensor_copy(zt[:], zt_ps[:])
        dzt = constp.tile([128, OL * NS], F32)
        nc.vector.tensor_sub(
            dzt[:, 0 : OL * NS - 1], zt[:, 1 : OL * NS], zt[:, 0 : OL * NS - 1]
        )

        outs = outp.tile([128, OL * I], F32)
        tl_ap = pk[:, _TL0 : _TL0 + I]

        for g in range(NGRP):
            for oi in range(GROUP):
                o = g * GROUP + oi
                col = o * I
                zc = NS * o
                # last row of each group (after group 0) runs on ScalarE so
                # both engines chew on the expansion in parallel
                on_act = oi == GROUP - 1 and g >= 1
                for j, t0, t1 in _SPANS:
                    if on_act:
                        nc.scalar.activation(
                            outs[:, col + t0 : col + t1],
                            tl_ap[:, t0:t1],
                            mybir.ActivationFunctionType.Identity,
                            bias=zt[:, zc + j : zc + j + 1],
                            scale=dzt[:, zc + j : zc + j + 1],
                        )
                    else:
                        nc.vector.tensor_scalar(
                            outs[:, col + t0 : col + t1],
                            tl_ap[:, t0:t1],
                            dzt[:, zc + j : zc + j + 1],
                            zt[:, zc + j : zc + j + 1],
                            mybir.AluOpType.mult,
                            mybir.AluOpType.add,
                        )
            nc.sync.dma_start(
                out_d[g * GROUP : (g + 1) * GROUP].rearrange("o b t -> b o t"),
                outs[:, g * GROUP * I : (g + 1) * GROUP * I].rearrange(
                    "p (o t) -> p o t", o=GROUP
                ),
            )

    nc.compile()
    return nc


def _get_nc():
    if "nc" not in _cache:
        _cache["nc"] = _build_nc()
    return _cache["nc"]


def _pack_inputs(x, control_values, expansion_matrix):
    x = np.ascontiguousarray(x, dtype=np.float32)
    cv = np.ascontiguousarray(control_values, dtype=np.float32)
    E = np.ascontiguousarray(expansion_matrix, dtype=np.float32)

    base = np.empty((128, _TOT), dtype=np.float32)
    for k in range(4):
        base[:, _XT0 + k * B : _XT0 + (k + 1) * B] = x[:, k * 128 : (k + 1) * 128].T
        base[:, _ET0 + k * C : _ET0 + (k + 1) * C] = E[:, k * 128 : (k + 1) * 128].T
    base[:, _TL0 : _TL0 + I] = _TL[None, :]

    in_maps = []
    for core in range(N_CORES):
        m = base.copy()
        slab = cv[core * OL : (core + 1) * OL].reshape(OL * NS, C)  # [(o,s), c]
        m[:, _CV0 : _CV0 + OL * NS] = slab.T
        in_maps.append({"pk": m})
    return in_maps


def _run(in_maps, trace=False):
    nc = _get_nc()
    return run_bass_kernel_spmd(
        nc, in_maps, core_ids=list(range(N_CORES)), trace=trace
    )


def kernel(x, control_points, control_values, expansion_matrix):
    in_maps = _pack_inputs(x, control_values, expansion_matrix)
    res = _run(in_maps, trace=False)
    return np.concatenate([r["out"] for r in res.results], axis=0)


def kernel_traced(x, control_points, control_values, expansion_matrix):
    """Same as kernel() but profiles on HW; returns (out, BassKernelResults)."""
    in_maps = _pack_inputs(x, control_values, expansion_matrix)
    res = _run(in_maps, trace=True)
    out = np.concatenate([r["out"] for r in res.results], axis=0)
    return out, res


# revision 2
# speedup vs baseline: 1.0743x; 1.0743x over previous
"""Trainium2 Bass kernel for nn_CorrectSplineLinear (embedding_lookup regime).

Math: reference computes
    W[o,t,f] = sum_c interp[o,t,c] * E[c,f]        (interp = piecewise-linear in t)
    out[o,b,t] = sum_f x[b,f] * W[o,t,f]
which collapses algebraically to
    y[c,b]    = sum_f E[c,f] * x[b,f]              ([128,128] matmul)
    Z[o,s,b]  = sum_c cv[o,s,c] * y[c,b]           ([128,128] matmul per core)
    out[o,b,t]= Z[o,j(t),b] + tl(t)*(Z[o,j(t)+1,b] - Z[o,j(t),b])
so no [O,I,I] weight is ever materialized.  The kernel is memory-bound on
writing the [256,128,512] fp32 output (8 MiB per core across 8 cores).

Sharding: out_features O=256 split across 8 cores (32 rows each); x and E
replicated; each core gets its control_values slice pre-transposed.

The expansion (one tensor_scalar/activation per spline segment per output
row: out = tl*dZ + Z with two per-partition scalars) is spread across
VectorE, ScalarE, and GpSimdE so the output DMA stream, not compute, is
the pacing resource.
"""

import sys
from contextlib import ExitStack

import numpy as np

try:
    import concourse.bass as bass
except ImportError:  # fresh grading dir: concourse lives in the repo checkout
    sys.path.insert(0, "/opt/trn_rl_repo")
    import concourse.bass as bass

import concourse.bacc as bacc
import concourse.mybir as mybir
import concourse.tile as tile
from concourse.bass_utils import run_bass_kernel_spmd

N_CORES = 8
O, I, K, C, B = 256, 512, 3, 128, 128
OL = O // N_CORES  # 32 output rows per core
NS = K + 1  # 4 control values per output row
F32 = mybir.dt.float32

# ---- spline geometry (input-independent, mirrors reference arithmetic) ----
_t = np.linspace(0.0, 1.0, I).astype(np.float32)
_ts = (_t * np.float32(K)).astype(np.float32)
_j = np.clip(np.floor(_ts), 0.0, float(K - 1)).astype(np.int32)
_TL = (_ts - _j.astype(np.float32)).astype(np.float32)  # [I] local coord in segment
_b0 = int(np.searchsorted(_j, 1))  # first t index in segment 1
_b1 = int(np.searchsorted(_j, 2))  # first t index in segment 2
# Even-length op spans for the DVE 2x fp32 perf mode.  Padded cells are
# overwritten by the middle-segment op, which is issued last on the same
# engine, so the final contents are exact.
_b0e = _b0 + (_b0 & 1)  # segment-0 op covers [0, _b0e)
_s2e = _b1 - (_b1 & 1)  # segment-2 op covers [_s2e, I)
_SPANS = [(0, 0, _b0e), (2, _s2e, I), (1, _b0, _b1)]  # (segment j, t0, t1)

# ---- packed-input column layout ([128, _TOT] fp32) ----
# 4 chunk-pairs [xT_k | eT_k] so matmul k can start as soon as chunk k lands,
# then cvT, then tl.
_CH0 = 0  # chunk k at [k*256, k*256+256): xT_k cols 0:128, eT_k cols 128:256
_CV0 = 4 * (B + C)  # cv slab transposed: [c, o*4+s]
_TL0 = _CV0 + OL * NS  # tl broadcast to 128 partitions
_TOT = _TL0 + I

GROUP = 4  # output rows per store DMA (4*128*512*4B = 1 MiB)
NGRP = OL // GROUP

# engine per output row: v=VectorE, a=ScalarE, g=GpSimdE
_ENG = []
for _g in range(NGRP):
    _ENG += ["v", "v", "a", "g"] if _g < 6 else ["v", "a", "a", "g"]

_cache: dict = {}


def _build_nc():
    nc = bacc.Bacc("TRN2", target_bir_lowering=False, debug=False, num_devices=N_CORES)
    pk_d = nc.dram_tensor("pk", [128, _TOT], F32, kind="ExternalInput")
    out_d = nc.dram_tensor("out", [OL, B, I], F32, kind="ExternalOutput")

    with tile.TileContext(nc) as tc, ExitStack() as ctx:
        constp = ctx.enter_context(tc.tile_pool(name="const", bufs=1))
        psump = ctx.enter_context(
            tc.tile_pool(name="psum", bufs=1, space=bass.MemorySpace.PSUM)
        )
        outp = ctx.enter_context(tc.tile_pool(name="outs", bufs=1))

        pk = constp.tile([128, _TOT], F32)
        # chunked input loads: matmul k only waits for its own 128KB
        for k in range(4):
            nc.sync.dma_start(
                pk[:, k * 256 : (k + 1) * 256], pk_d[:, k * 256 : (k + 1) * 256]
            )
        nc.sync.dma_start(pk[:, _CV0:_TL0], pk_d[:, _CV0:_TL0])
        nc.sync.dma_start(pk[:, _TL0:_TOT], pk_d[:, _TL0:_TOT])

        # y[c,b] = sum_f E[c,f] x[b,f]: accumulate over 4 chunks of f.
        y_ps = psump.tile([128, B], F32)
        for k in range(4):
            base = k * 256
            nc.tensor.matmul(
                y_ps[:],
                pk[:, base + B : base + B + C],  # lhsT [f_chunk, c]
                pk[:, base : base + B],  # rhs  [f_chunk, b]
                start=(k == 0),
                stop=(k == 3),
            )
        y_sb = constp.tile([128, B], F32)
        # ScalarE Identity == copy; same ACT table set as the expansion ops
        nc.scalar.activation(y_sb[:], y_ps[:], mybir.ActivationFunctionType.Identity)

        # ZT[b, o*4+s] = sum_c y[c,b] cvT[c, o*4+s]
        zt_ps = psump.tile([128, OL * NS], F32)
        nc.tensor.matmul(
            zt_ps[:], y_sb[:], pk[:, _CV0 : _CV0 + OL * NS], start=True, stop=True
        )
        zt = constp.tile([128, OL * NS], F32)
        nc.vector.tensor_copy(zt[:], zt_ps[:])
        dzt = constp.tile([128, OL * NS], F32)
        nc.vector.tensor_sub(
            dzt[:, 0 : OL * NS - 1], zt[:, 1 : OL * NS], zt[:, 0 : OL * NS - 1]
        )

        outs = outp.tile([128, OL * I], F32)
        tl_ap = pk[:, _TL0 : _TL0 + I]

        for g in range(NGRP):
            for oi in range(GROUP):
                o = g * GROUP + oi
                col = o * I
                zc = NS * o
                eng = _ENG[o]
                for j, t0, t1 in _SPANS:
                    if eng == "a":
                        nc.scalar.activation(
                            outs[:, col + t0 : col + t1],
                            tl_ap[:, t0:t1],
                            mybir.ActivationFunctionType.Identity,
                            bias=zt[:, zc + j : zc + j + 1],
                            scale=dzt[:, zc + j : zc + j + 1],
                        )
                    else:
                        veng = nc.vector if eng == "v" else nc.gpsimd
                        veng.tensor_scalar(
                            outs[:, col + t0 : col + t1],
                            tl_ap[:, t0:t1],
                            dzt[:, zc + j : zc + j + 1],
                            zt[:, zc + j : zc + j + 1],
                            mybir.AluOpType.mult,
                            mybir.AluOpType.add,
                        )
            if g == 0:
                # two 512KB stores so the HBM write stream starts ASAP
                for h in range(2):
                    o0 = h * 2
                    nc.sync.dma_start(
                        out_d[o0 : o0 + 2].rearrange("o b t -> b o t"),
                        outs[:, o0 * I : (o0 + 2) * I].rearrange(
                            "p (o t) -> p o t", o=2
                        ),
                    )
            else:
                nc.sync.dma_start(
                    out_d[g * GROUP : (g + 1) * GROUP].rearrange("o b t -> b o t"),
                    outs[:, g * GROUP * I : (g + 1) * GROUP * I].rearrange(
                        "p (o t) -> p o t", o=GROUP
                    ),
                )

    nc.compile()
    return nc


def _get_nc():
    if "nc" not in _cache:
        _cache["nc"] = _build_nc()
    return _cache["nc"]


def _pack_inputs(x, control_values, expansion_matrix):
    x = np.ascontiguousarray(x, dtype=np.float32)
    cv = np.ascontiguousarray(control_values, dtype=np.float32)
    E = np.ascontiguousarray(expansion_matrix, dtype=np.float32)

    base = np.empty((128, _TOT), dtype=np.float32)
    for k in range(4):
        base[:, k * 256 : k * 256 + B] = x[:, k * 128 : (k + 1) * 128].T
        base[:, k * 256 + B : k * 256 + B + C] = E[:, k * 128 : (k + 1) * 128].T
    base[:, _TL0 : _TL0 + I] = _TL[None, :]

    in_maps = []
    for core in range(N_CORES):
        m = base.copy()
        slab = cv[core * OL : (core + 1) * OL].reshape(OL * NS, C)  # [(o,s), c]
        m[:, _CV0 : _CV0 + OL * NS] = slab.T
        in_maps.append({"pk": m})
    return in_maps


def _run(in_maps, trace=False):
    nc = _get_nc()
    return run_bass_kernel_spmd(
        nc, in_maps, core_ids=list(range(N_CORES)), trace=trace
    )


def kernel(x, control_points, control_values, expansion_matrix):
    in_maps = _pack_inputs(x, control_values, expansion_matrix)
    res = _run(in_maps, trace=False)
    return np.concatenate([r["out"] for r in res.results], axis=0)


def kernel_traced(x, control_points, control_values, expansion_matrix):
    """Same as kernel() but profiles on HW; returns (out, BassKernelResults)."""
    in_maps = _pack_inputs(x, control_values, expansion_matrix)
    res = _run(in_maps, trace=True)
    out = np.concatenate([r["out"] for r in res.results], axis=0)
    return out, res
